# revision 1
# baseline (speedup 1.0000x reference)
"""Chebyshev self-attention Trainium2 kernel (8-core SPMD).

Math restructuring
------------------
reference:  scores = (q @ k.T)/8 + cheb_bias(alphas)[h]  ;  softmax ; @ v

The Chebyshev relative-position bias T_k((j-i)/(S-1)) is a degree-5
polynomial in (u_j - v_i) with u=j/(S-1), v=i/(S-1), so it factors exactly
as a rank-6 term:

    bias_h[i, j] = sum_b G_h[i, b] * u_j**b           (b = 0..5)

with G_h[i, b] = sum_a c_h[a+b] * C(a+b, a) * (-v_i)**a and c_h the
monomial coefficients of sum_k alphas[h,k] T_k.  G depends on the runtime
input `alphas`, so it is computed on the host (tiny: [H, 6, S]) and fed to
the device, where it rides along as 6 extra contraction rows of the QK
matmul (K = 64 + 6 = 70).  No [S, S] bias tensor is ever materialized.

Attention is computed in transposed orientation per head:
    scoresT[j, i] = kaugT.T @ qaugT   (j on partitions)
    expT = exp(scoresT)               (no max-subtraction; |scores| <~ 3)
    ctxT[d, i]   = sum_j v_aug[j, d] * expT[j, i]     (PSUM accumulation)
where v_aug has a 65th column of ones so row 64 of ctxT is the softmax
denominator.  ctxT tiles are PE-transposed back to [i, d], scaled by the
reciprocal denominator, and DMA'd out.  probs never need a transpose.

Sharding: core = batch (4) x head-group (2 x 6 heads).  Weights are
pre-transposed/sliced on the host; 1/sqrt(64) is folded into Wq.
Biases: bv is exact via host-add after (sum probs = 1); bq/bk are folded
into the PSUM->SBUF epilogue copies as per-partition activation biases.
"""

import numpy as np
from math import comb

B = 4
S = 2048
HIDDEN = 768
HEADS = 12
D = 64
ORDER = 5
NCORES = 8
HG = HEADS // 2          # heads per core (6)
HGDIM = HG * D           # 384 output columns per core
CC = HIDDEN // 128       # contraction chunks (6)
JC = S // 128            # j tiles (16)
IC = S // 128            # i tiles (16)
KAUG = D + ORDER + 1     # 70

_CACHE = {}


def _cheb_factors(alphas: np.ndarray):
    """alphas [H, 6] -> G [H, 6, S] (i-side, f32), P [6, S] (j-side, f32)."""
    import numpy.polynomial.chebyshev as cheb

    T = np.zeros((ORDER + 1, ORDER + 1))
    for k in range(ORDER + 1):
        e = np.zeros(k + 1)
        e[k] = 1
        T[k, : k + 1] = cheb.cheb2poly(e)[: k + 1]
    c = alphas.astype(np.float64) @ T              # [H, 6] monomial coeffs
    v = np.arange(S, dtype=np.float64) / (S - 1)
    G = np.zeros((HEADS, ORDER + 1, S))
    for h in range(HEADS):
        for b in range(ORDER + 1):
            acc = np.zeros(S)
            for a in range(0, ORDER + 1 - b):
                acc += c[h, a + b] * comb(a + b, a) * ((-v) ** a)
            G[h, b, :] = acc
    P = np.stack([v**b for b in range(ORDER + 1)], 0)  # [6, S]
    return G.astype(np.float32), P.astype(np.float32)


def _build_program(use_qk_bias: bool):
    import concourse.bass as bass
    import concourse.mybir as mybir
    import concourse.tile as tile
    from concourse import bacc

    f32 = mybir.dt.float32
    nc = bacc.Bacc("TRN2", target_bir_lowering=False, debug=False)

    bf16 = mybir.dt.bfloat16
    hsT_d = nc.dram_tensor("hsT", [HIDDEN, S], bf16, kind="ExternalInput")
    w_d = nc.dram_tensor("w", [3, HIDDEN, HGDIM], bf16, kind="ExternalInput")
    g_d = nc.dram_tensor("g", [HG, ORDER + 1, S], f32, kind="ExternalInput")
    bqk_d = nc.dram_tensor("bqk", [2, HGDIM], f32, kind="ExternalInput")
    out_d = nc.dram_tensor("out", [S, HGDIM], f32, kind="ExternalOutput")

    pT_np = np.stack(
        [(np.arange(S, dtype=np.float64) / (S - 1)) ** b for b in range(ORDER + 1)], 0
    ).astype(np.float32)
    pT_d = nc.inline_tensor(pT_np, name="pT")
    ones_d = nc.inline_tensor(np.ones((JC, HG), np.float32), name="ones")
    ident_d = nc.inline_tensor(np.eye(128, dtype=np.float32), name="ident")

    Exp = mybir.ActivationFunctionType.Exp
    Ident = mybir.ActivationFunctionType.Identity
    f32r = mybir.dt.float32r

    def r(ap):
        # float32r: same 4-byte data, PE runs the relaxed-precision multiply
        # path at 1 cycle/row (vs 4 for strict fp32).
        return ap.bitcast(f32r)

    with tile.TileContext(nc) as tc:
        import contextlib

        with contextlib.ExitStack() as ctx:
            consts = ctx.enter_context(tc.tile_pool(name="consts", bufs=1))
            # per-chunk tiles so compute can start as soon as each DMA lands
            hsT = [consts.tile([128, S], bf16, name=f"hsT{cc}") for cc in range(CC)]
            w_sb = [
                [consts.tile([128, HGDIM], bf16, name=f"w{t}_{cc}") for cc in range(CC)]
                for t in range(3)
            ]
            for cc in range(CC):
                nc.sync.dma_start(out=hsT[cc][:], in_=hsT_d[cc * 128 : (cc + 1) * 128, :])
                nc.sync.dma_start(
                    out=w_sb[2][cc][:], in_=w_d[2, cc * 128 : (cc + 1) * 128, :]
                )
            for t in range(2):
                for cc in range(CC):
                    nc.sync.dma_start(
                        out=w_sb[t][cc][:], in_=w_d[t, cc * 128 : (cc + 1) * 128, :]
                    )
            ident = consts.tile([128, 128], f32)
            nc.sync.dma_start(out=ident[:], in_=ident_d[:])
            if use_qk_bias:
                bqk = consts.tile([128, 2, 3], f32)
                nc.sync.dma_start(
                    out=bqk[:],
                    in_=bqk_d.ap().rearrange("t (hp p) -> p t hp", p=128),
                )
            v_sb = consts.tile([128, JC, HG * (D + 1)], f32r)
            v4 = v_sb.rearrange("p jc (h x) -> p jc h x", x=D + 1)
            ones_ap = ones_d.ap()
            ones_bcast = bass.AP(
                tensor=ones_ap.tensor,
                offset=ones_ap.offset,
                ap=[[0, 128]] + list(ones_ap.ap),
            )
            nc.sync.dma_start(out=v4[:, :, :, D], in_=r(ones_bcast))

            # ---- V projection: v[j, d] for all 6 heads ----
            with tc.tile_pool(name="vpsum", bufs=3, space="PSUM") as vpsum:
                for jc in range(JC):
                    vp = vpsum.tile([128, HGDIM], f32)
                    for cc in range(CC):
                        nc.tensor.matmul(
                            vp[:],
                            hsT[cc][:, jc * 128 : (jc + 1) * 128],
                            w_sb[2][cc][:],
                            start=(cc == 0),
                            stop=(cc == CC - 1),
                        )
                    nc.vector.tensor_copy(v4[:, jc, :, 0:D], vp.rearrange("p (h d) -> p h d", d=D))

            aug = ctx.enter_context(tc.tile_pool(name="aug", bufs=2))
            psp = ctx.enter_context(tc.tile_pool(name="psp", bufs=4, space="PSUM"))
            expp = ctx.enter_context(tc.tile_pool(name="expp", bufs=4))
            ctxs = ctx.enter_context(tc.tile_pool(name="ctxs", bufs=2))
            outs = ctx.enter_context(tc.tile_pool(name="outs", bufs=2))
            small = ctx.enter_context(tc.tile_pool(name="small", bufs=8))

            aug_tiles = {}

            def make_aug(hp):
                qe = aug.tile([128, S], f32r, tag="qaug_e")
                qo = aug.tile([128, S], f32r, tag="qaug_o")
                ke = aug.tile([128, S], f32r, tag="kaug_e")
                ko = aug.tile([128, S], f32r, tag="kaug_o")
                nc.sync.dma_start(out=qe[D : D + 6, :], in_=r(g_d[2 * hp, :, :]))
                nc.sync.dma_start(out=qo[D : D + 6, :], in_=r(g_d[2 * hp + 1, :, :]))
                nc.sync.dma_start(out=ke[D : D + 6, :], in_=r(pT_d[:]))
                nc.sync.dma_start(out=ko[D : D + 6, :], in_=r(pT_d[:]))
                aug_tiles[hp] = (qe, qo, ke, ko)

            def proj_thunks(hp, t, half):
                """One (tensor, i-half) projection as 7 small thunks: 6 cc-step
                matmul pairs into a shared PSUM tile + an epilogue copy."""
                qe, qo, ke, ko = aug_tiles[hp]
                dst_e, dst_o = (qe, qo) if t == 0 else (ke, ko)
                fs = slice(half * 1024, (half + 1) * 1024)
                st = {}

                def step(cc):
                    def f():
                        if cc == 0:
                            st["pp"] = psp.tile([128, 1024], f32, tag="ps", name="pp")
                        pp = st["pp"]
                        for nn in range(2):
                            ns = slice(half * 1024 + nn * 512, half * 1024 + (nn + 1) * 512)
                            nc.tensor.matmul(
                                pp[:, nn * 512 : (nn + 1) * 512],
                                w_sb[t][cc][:, hp * 128 : (hp + 1) * 128],
                                hsT[cc][:, ns],
                                start=(cc == 0),
                                stop=(cc == CC - 1),
                            )
                    return f

                def fin():
                    pp = st["pp"]
                    if use_qk_bias:
                        nc.scalar.activation(dst_e[0:D, fs], pp[0:D, :], Ident, bias=bqk[0:D, t, hp])
                        nc.scalar.activation(dst_o[0:D, fs], pp[D:128, :], Ident, bias=bqk[D:128, t, hp])
                    else:
                        nc.vector.tensor_copy(dst_e[0:D, fs], pp[0:D, :])
                        nc.vector.tensor_copy(dst_o[0:D, fs], pp[D:128, :])

                return [step(cc) for cc in range(CC)] + [fin]

            def tail_thunks(h, ctx_sb):
                """Per-i-chunk transpose + reciprocal-normalize + final DMA out,
                as 17 thunks to spread into later sections' jc loops."""
                st = {}

                def mk(ic):
                    def f():
                        if "o" not in st:
                            st["o"] = outs.tile([128, IC, D], f32, tag="out_h", name="out_h")
                        tp = psp.tile([128, 1024], f32, tag="ps", name="tp")
                        nc.tensor.transpose(
                            tp[:, 0 : D + 1],
                            ctx_sb[0 : D + 1, ic * 128 : (ic + 1) * 128],
                            ident[0 : D + 1, 0 : D + 1],
                        )
                        rv = small.tile([128, 1], f32, tag="rv")
                        nc.vector.reciprocal(rv[:], tp[:, D : D + 1])
                        nc.vector.tensor_scalar_mul(st["o"][:, ic, :], tp[:, 0:D], rv[:])
                    return f

                def fin():
                    nc.sync.dma_start(
                        out=out_d.ap().rearrange("(ic p) (h d) -> p ic h d", p=128, d=D)[:, :, h, :],
                        in_=st["o"][:],
                    )

                return [mk(ic) for ic in range(IC)] + [fin]

            def attn_section(h, ihalf, ctx_sb, fillq, prev_defer):
                """16-jc flash loop for one (head, i-half).  ctx matmuls lag one
                jc behind scores; the last two ctx matmuls + the PSUM->SBUF copy
                are DEFERRED into the next section's first iterations so the PE
                stream crosses section boundaries without draining ACT.
                Returns this section's deferred thunks."""
                hp, par = divmod(h, 2)
                qa = aug_tiles[hp][par]
                ka = aug_tiles[hp][2 + par]
                st = {}
                ets = [None] * JC

                def emit_ctx(j):
                    if j == 0:
                        st["cp"] = psp.tile([128, 1024], f32, tag="ps", name="cp")
                    cp = st["cp"]
                    for nn in range(2):
                        nc.tensor.matmul(
                            cp[0 : D + 1, nn * 512 : (nn + 1) * 512],
                            v_sb[:, j, h * (D + 1) : (h + 1) * (D + 1)],
                            ets[j][:, nn * 512 : (nn + 1) * 512],
                            start=(j == 0),
                            stop=(j == JC - 1),
                        )

                def final_copy():
                    nc.vector.tensor_copy(
                        ctx_sb[0 : D + 1, ihalf * 1024 : (ihalf + 1) * 1024],
                        st["cp"][0 : D + 1, :],
                    )

                for jc in range(JC):
                    sp = psp.tile([128, 1024], f32, tag="ps", name="sp")
                    for nn in range(2):
                        nc.tensor.matmul(
                            sp[:, nn * 512 : (nn + 1) * 512],
                            ka[0:KAUG, jc * 128 : (jc + 1) * 128],
                            qa[0:KAUG, ihalf * 1024 + nn * 512 : ihalf * 1024 + (nn + 1) * 512],
                            start=True,
                            stop=True,
                        )
                    et = expp.tile([128, 1024], f32r, tag="expp")
                    nc.scalar.activation(et[:], sp[:], Exp)
                    ets[jc] = et
                    if jc < len(prev_defer):
                        prev_defer[jc]()
                    # own lagged ctx, one jc behind; the last two are deferred
                    if jc > 0 and jc - 1 <= JC - 3:
                        emit_ctx(jc - 1)
                    if jc >= 2 and fillq:
                        fillq.pop(0)()
                for th in fillq:
                    th()
                return [
                    (lambda: emit_ctx(JC - 2)),
                    (lambda: (emit_ctx(JC - 1), final_copy())),
                ]

            # ---- pair 0 projections: q-half1 deferred into the first section ----
            make_aug(0)
            for th in proj_thunks(0, 0, 0) + proj_thunks(0, 1, 0) + proj_thunks(0, 1, 1):
                th()

            ctx_sbs = {}
            defer = []
            pending_tail = []  # tail thunks of the previous pair's odd head
            carry = proj_thunks(0, 0, 1)  # q half-1: needed only from (h0, ihalf1)
            for hp in range(3):
                if hp + 1 < 3:
                    make_aug(hp + 1)
                    pj = []
                    for t in range(2):
                        for half in range(2):
                            pj += proj_thunks(hp + 1, t, half)
                else:
                    pj = []
                h_e, h_o = 2 * hp, 2 * hp + 1
                ctx_sbs[h_e] = ctxs.tile([D + 1, S], f32, tag="ctx_sb", name="ctx_sb")
                ctx_sbs[h_o] = ctxs.tile([D + 1, S], f32, tag="ctx_sb", name="ctx_sb")

                s0_fill = (carry + pending_tail)[:16]
                rest = (carry + pending_tail)[16:]
                carry = []
                s1_fill = rest + pj[: 16 - len(rest)]
                pj = pj[16 - len(rest) :]

                defer = attn_section(h_e, 0, ctx_sbs[h_e], s0_fill, defer)
                defer = attn_section(h_e, 1, ctx_sbs[h_e], s1_fill, defer)

                te = tail_thunks(h_e, ctx_sbs[h_e])
                s2_fill = te[:16]
                s3_fill = te[16:] + pj
                defer = attn_section(h_o, 0, ctx_sbs[h_o], s2_fill, defer)
                defer = attn_section(h_o, 1, ctx_sbs[h_o], s3_fill, defer)

                pending_tail = tail_thunks(h_o, ctx_sbs[h_o])

            for th in defer:
                th()
            for th in pending_tail:
                th()

    nc.finalize()
    return nc

def kernel(hidden_states, Wq, bq, Wk, bk, Wv, bv, alphas):
    from concourse.bass_utils import run_bass_kernel_spmd

    hidden_states = np.asarray(hidden_states, dtype=np.float32)
    Wq = np.asarray(Wq, dtype=np.float32)
    Wk = np.asarray(Wk, dtype=np.float32)
    Wv = np.asarray(Wv, dtype=np.float32)
    bq = np.asarray(bq, dtype=np.float32)
    bk = np.asarray(bk, dtype=np.float32)
    bv = np.asarray(bv, dtype=np.float32)
    alphas = np.asarray(alphas, dtype=np.float32)

    use_qk_bias = bool(np.any(bq) or np.any(bk))
    key = ("prog", use_qk_bias)
    if key not in _CACHE:
        _CACHE[key] = _build_program(use_qk_bias)
    nc = _CACHE[key]

    G, _ = _cheb_factors(alphas)  # [12, 6, S]
    scale = 1.0 / np.sqrt(np.float32(D)).astype(np.float32)

    in_maps = []
    for core in range(NCORES):
        b, hg = divmod(core, 2)
        rows = slice(hg * HGDIM, (hg + 1) * HGDIM)
        hsT = np.ascontiguousarray(hidden_states[b].T)  # [768, S]
        w = np.stack(
            [
                np.ascontiguousarray(Wq[rows, :].T) * scale,
                np.ascontiguousarray(Wk[rows, :].T),
                np.ascontiguousarray(Wv[rows, :].T),
            ],
            0,
        )  # [3, 768, 384]
        g = np.ascontiguousarray(G[hg * HG : (hg + 1) * HG])  # [6, 6, S]
        bqk = np.stack([bq[rows] * scale, bk[rows]], 0)  # [2, 384]
        import ml_dtypes

        in_maps.append(
            {
                "hsT": hsT.astype(ml_dtypes.bfloat16),
                "w": w.astype(ml_dtypes.bfloat16),
                "g": g.astype(np.float32),
                "bqk": np.ascontiguousarray(bqk, dtype=np.float32),
            }
        )

    res = run_bass_kernel_spmd(nc, in_maps, list(range(NCORES)))

    out = np.empty((B, S, HIDDEN), dtype=np.float32)
    for core in range(NCORES):
        b, hg = divmod(core, 2)
        o = res.results[core]["out"]  # [S, 384]
        if np.any(bv):
            o = o + bv[hg * HGDIM : (hg + 1) * HGDIM][None, :]
        out[b, :, hg * HGDIM : (hg + 1) * HGDIM] = o
    return out



# revision 4
# speedup vs baseline: 1.0398x; 1.0398x over previous
"""Chebyshev self-attention Trainium2 kernel (8-core SPMD).

Math restructuring
------------------
reference:  scores = (q @ k.T)/8 + cheb_bias(alphas)[h]  ;  softmax ; @ v

The Chebyshev relative-position bias T_k((j-i)/(S-1)) is a degree-5
polynomial in (u_j - v_i) with u=j/(S-1), v=i/(S-1), so it factors exactly
as a rank-6 term:

    bias_h[i, j] = sum_b G_h[i, b] * u_j**b           (b = 0..5)

with G_h[i, b] = sum_a c_h[a+b] * C(a+b, a) * (-v_i)**a and c_h the
monomial coefficients of sum_k alphas[h,k] T_k.  G depends on the runtime
input `alphas`, so it is computed on the host (tiny: [H, 6, S]) and fed to
the device, where it rides along as 6 extra contraction rows of the QK
matmul (K = 64 + 6 = 70).  No [S, S] bias tensor is ever materialized.

Attention is computed in transposed orientation per head:
    scoresT[j, i] = kaugT.T @ qaugT   (j on partitions)
    expT = exp(scoresT)               (no max-subtraction; |scores| <~ 3)
    ctxT[d, i]   = sum_j v_aug[j, d] * expT[j, i]     (PSUM accumulation)
where v_aug has a 65th column of ones so row 64 of ctxT is the softmax
denominator.  ctxT tiles are PE-transposed back to [i, d], scaled by the
reciprocal denominator, and DMA'd out.  probs never need a transpose.

Sharding: core = batch (4) x head-group (2 x 6 heads).  Weights are
pre-transposed/sliced on the host; 1/sqrt(64) is folded into Wq.
Biases: bv is exact via host-add after (sum probs = 1); bq/bk are folded
into the PSUM->SBUF epilogue copies as per-partition activation biases.

Dispatch-overhead engineering (the axon tunnel costs ~0.7 ms per NEFF
argument and ~1.2 ms per MB-per-core of output):
  * all per-core inputs are packed into ONE 1-D bf16 "blob" tensor
    (f32 sections ride as raw byte pairs and are DMA'd into f32 SBUF
    tiles through a width-changing bitcast on the destination AP);
  * the output is bf16 (the f32->bf16 rounding adds ~1e-3 relative
    error against a 2e-2 budget); the host converts back and adds bv;
  * the PJRT executable + donated-zero output buffers are built once and
    cached, so repeat kernel() calls only upload the blob and dispatch.
"""

import numpy as np
from math import comb

B = 4
S = 2048
HIDDEN = 768
HEADS = 12
D = 64
ORDER = 5
NCORES = 8
HG = HEADS // 2          # heads per core (6)
HGDIM = HG * D           # 384 output columns per core
CC = HIDDEN // 128       # contraction chunks (6)
JC = S // 128            # j tiles (16)
IC = S // 128            # i tiles (16)
KAUG = D + ORDER + 1     # 70

# blob layout (bf16 element offsets, per core)
OFF_HS = 0
N_HS = HIDDEN * S                      # [768, 2048] bf16
OFF_W = OFF_HS + N_HS
N_W = 3 * HIDDEN * HGDIM               # [3, 768, 384] bf16
OFF_G = OFF_W + N_W
N_G = HG * (ORDER + 1) * S * 2         # [6, 6, 2048] f32 as bf16 pairs
OFF_BQK = OFF_G + N_G
N_BQK = 2 * HGDIM * 2                  # [2, 384] f32 as bf16 pairs
NB_NOBIAS = OFF_BQK
NB_BIAS = OFF_BQK + N_BQK

_CACHE = {}


def _cheb_factors(alphas: np.ndarray):
    """alphas [H, 6] -> G [H, 6, S] (i-side, f32), P [6, S] (j-side, f32)."""
    import numpy.polynomial.chebyshev as cheb

    T = np.zeros((ORDER + 1, ORDER + 1))
    for k in range(ORDER + 1):
        e = np.zeros(k + 1)
        e[k] = 1
        T[k, : k + 1] = cheb.cheb2poly(e)[: k + 1]
    c = alphas.astype(np.float64) @ T              # [H, 6] monomial coeffs
    v = np.arange(S, dtype=np.float64) / (S - 1)
    G = np.zeros((HEADS, ORDER + 1, S))
    for h in range(HEADS):
        for b in range(ORDER + 1):
            acc = np.zeros(S)
            for a in range(0, ORDER + 1 - b):
                acc += c[h, a + b] * comb(a + b, a) * ((-v) ** a)
            G[h, b, :] = acc
    P = np.stack([v**b for b in range(ORDER + 1)], 0)  # [6, S]
    return G.astype(np.float32), P.astype(np.float32)


def _build_program(use_qk_bias: bool):
    import concourse.bass as bass
    import concourse.mybir as mybir
    import concourse.tile as tile
    from concourse import bacc

    f32 = mybir.dt.float32
    nc = bacc.Bacc("TRN2", target_bir_lowering=False, debug=False)

    bf16 = mybir.dt.bfloat16
    nb = NB_BIAS if use_qk_bias else NB_NOBIAS
    blob_d = nc.dram_tensor("blob", [nb], bf16, kind="ExternalInput")
    out_d = nc.dram_tensor("out", [S, HGDIM], bf16, kind="ExternalOutput")

    blob = blob_d.ap()
    hsT_d = blob[OFF_HS : OFF_HS + N_HS].rearrange("(p s) -> p s", s=S)
    w_d = blob[OFF_W : OFF_W + N_W].rearrange("(t p n) -> t p n", p=HIDDEN, n=HGDIM)
    # f32 payload viewed as bf16 pairs: [6 heads, 6 coeffs, 2*S]
    g_d = blob[OFF_G : OFF_G + N_G].rearrange("(h k s) -> h k s", k=ORDER + 1, s=2 * S)

    pT_np = np.stack(
        [(np.arange(S, dtype=np.float64) / (S - 1)) ** b for b in range(ORDER + 1)], 0
    ).astype(np.float32)
    pT_d = nc.inline_tensor(pT_np, name="pT")
    ones_d = nc.inline_tensor(np.ones((JC, HG), np.float32), name="ones")
    ident_d = nc.inline_tensor(np.eye(128, dtype=np.float32), name="ident")

    Exp = mybir.ActivationFunctionType.Exp
    Ident = mybir.ActivationFunctionType.Identity
    f32r = mybir.dt.float32r

    def r(ap):
        # float32r: same 4-byte data, PE runs the relaxed-precision multiply
        # path at 1 cycle/row (vs 4 for strict fp32).
        return ap.bitcast(f32r)

    with tile.TileContext(nc) as tc:
        import contextlib

        with contextlib.ExitStack() as ctx:
            consts = ctx.enter_context(tc.tile_pool(name="consts", bufs=1))
            # per-chunk tiles so compute can start as soon as each DMA lands
            hsT = [consts.tile([128, S], bf16, name=f"hsT{cc}") for cc in range(CC)]
            w_sb = [
                [consts.tile([128, HGDIM], bf16, name=f"w{t}_{cc}") for cc in range(CC)]
                for t in range(3)
            ]
            for cc in range(CC):
                nc.sync.dma_start(out=hsT[cc][:], in_=hsT_d[cc * 128 : (cc + 1) * 128, :])
                nc.sync.dma_start(
                    out=w_sb[2][cc][:], in_=w_d[2, cc * 128 : (cc + 1) * 128, :]
                )
            for t in range(2):
                for cc in range(CC):
                    nc.sync.dma_start(
                        out=w_sb[t][cc][:], in_=w_d[t, cc * 128 : (cc + 1) * 128, :]
                    )
            ident = consts.tile([128, 128], f32)
            nc.sync.dma_start(out=ident[:], in_=ident_d[:])
            if use_qk_bias:
                bqk_d = blob[OFF_BQK : OFF_BQK + N_BQK].rearrange(
                    "(t hp p two) -> p t hp two", t=2, hp=3, p=128, two=2
                )
                bqk = consts.tile([128, 2, 3], f32)
                nc.sync.dma_start(out=bqk[:].bitcast(bf16), in_=bqk_d)
            v_sb = consts.tile([128, JC, HG * (D + 1)], f32r)
            v4 = v_sb.rearrange("p jc (h x) -> p jc h x", x=D + 1)
            ones_ap = ones_d.ap()
            ones_bcast = bass.AP(
                tensor=ones_ap.tensor,
                offset=ones_ap.offset,
                ap=[[0, 128]] + list(ones_ap.ap),
            )
            nc.sync.dma_start(out=v4[:, :, :, D], in_=r(ones_bcast))

            # ---- V projection: v[j, d] for all 6 heads ----
            with tc.tile_pool(name="vpsum", bufs=3, space="PSUM") as vpsum:
                for jc in range(JC):
                    vp = vpsum.tile([128, HGDIM], f32)
                    for cc in range(CC):
                        nc.tensor.matmul(
                            vp[:],
                            hsT[cc][:, jc * 128 : (jc + 1) * 128],
                            w_sb[2][cc][:],
                            start=(cc == 0),
                            stop=(cc == CC - 1),
                        )
                    nc.vector.tensor_copy(v4[:, jc, :, 0:D], vp.rearrange("p (h d) -> p h d", d=D))

            aug = ctx.enter_context(tc.tile_pool(name="aug", bufs=2))
            psp = ctx.enter_context(tc.tile_pool(name="psp", bufs=4, space="PSUM"))
            expp = ctx.enter_context(tc.tile_pool(name="expp", bufs=4))
            ctxs = ctx.enter_context(tc.tile_pool(name="ctxs", bufs=2))
            outs = ctx.enter_context(tc.tile_pool(name="outs", bufs=2))
            small = ctx.enter_context(tc.tile_pool(name="small", bufs=8))

            aug_tiles = {}

            def make_aug(hp):
                qe = aug.tile([128, S], f32r, tag="qaug_e")
                qo = aug.tile([128, S], f32r, tag="qaug_o")
                ke = aug.tile([128, S], f32r, tag="kaug_e")
                ko = aug.tile([128, S], f32r, tag="kaug_o")
                # g rows are f32 bytes stored as bf16 pairs in the blob; the
                # width-halving bitcast makes the DMA write f32r-typed (the BIR
                # verifier requires fp32r-matmul inputs to be written as f32r)
                nc.sync.dma_start(out=qe[D : D + 6, :], in_=g_d[2 * hp].bitcast(f32r))
                nc.sync.dma_start(out=qo[D : D + 6, :], in_=g_d[2 * hp + 1].bitcast(f32r))
                nc.sync.dma_start(out=ke[D : D + 6, :], in_=r(pT_d[:]))
                nc.sync.dma_start(out=ko[D : D + 6, :], in_=r(pT_d[:]))
                aug_tiles[hp] = (qe, qo, ke, ko)

            def proj_thunks(hp, t, half):
                """One (tensor, i-half) projection as 7 small thunks: 6 cc-step
                matmul pairs into a shared PSUM tile + an epilogue copy."""
                qe, qo, ke, ko = aug_tiles[hp]
                dst_e, dst_o = (qe, qo) if t == 0 else (ke, ko)
                fs = slice(half * 1024, (half + 1) * 1024)
                st = {}

                def step(cc):
                    def f():
                        if cc == 0:
                            st["pp"] = psp.tile([128, 1024], f32, tag="ps", name="pp")
                        pp = st["pp"]
                        for nn in range(2):
                            ns = slice(half * 1024 + nn * 512, half * 1024 + (nn + 1) * 512)
                            nc.tensor.matmul(
                                pp[:, nn * 512 : (nn + 1) * 512],
                                w_sb[t][cc][:, hp * 128 : (hp + 1) * 128],
                                hsT[cc][:, ns],
                                start=(cc == 0),
                                stop=(cc == CC - 1),
                            )
                    return f

                def fin():
                    pp = st["pp"]
                    if use_qk_bias:
                        nc.scalar.activation(dst_e[0:D, fs], pp[0:D, :], Ident, bias=bqk[0:D, t, hp])
                        nc.scalar.activation(dst_o[0:D, fs], pp[D:128, :], Ident, bias=bqk[D:128, t, hp])
                    else:
                        nc.vector.tensor_copy(dst_e[0:D, fs], pp[0:D, :])
                        nc.vector.tensor_copy(dst_o[0:D, fs], pp[D:128, :])

                return [step(cc) for cc in range(CC)] + [fin]

            def tail_thunks(h, ctx_sb):
                """Per-i-chunk transpose + reciprocal-normalize + final DMA out,
                as 17 thunks to spread into later sections' jc loops."""
                st = {}

                def mk(ic):
                    def f():
                        if "o" not in st:
                            st["o"] = outs.tile([128, IC, D], bf16, tag="out_h", name="out_h")
                        tp = psp.tile([128, 1024], f32, tag="ps", name="tp")
                        nc.tensor.transpose(
                            tp[:, 0 : D + 1],
                            ctx_sb[0 : D + 1, ic * 128 : (ic + 1) * 128],
                            ident[0 : D + 1, 0 : D + 1],
                        )
                        rv = small.tile([128, 1], f32, tag="rv")
                        nc.vector.reciprocal(rv[:], tp[:, D : D + 1])
                        nc.vector.tensor_scalar_mul(st["o"][:, ic, :], tp[:, 0:D], rv[:])
                    return f

                def fin():
                    nc.sync.dma_start(
                        out=out_d.ap().rearrange("(ic p) (h d) -> p ic h d", p=128, d=D)[:, :, h, :],
                        in_=st["o"][:],
                    )

                return [mk(ic) for ic in range(IC)] + [fin]

            def attn_section(h, ihalf, ctx_sb, fillq, prev_defer):
                """16-jc flash loop for one (head, i-half).  ctx matmuls lag one
                jc behind scores; the last two ctx matmuls + the PSUM->SBUF copy
                are DEFERRED into the next section's first iterations so the PE
                stream crosses section boundaries without draining ACT.
                Returns this section's deferred thunks."""
                hp, par = divmod(h, 2)
                qa = aug_tiles[hp][par]
                ka = aug_tiles[hp][2 + par]
                st = {}
                ets = [None] * JC

                def emit_ctx(j):
                    if j == 0:
                        st["cp"] = psp.tile([128, 1024], f32, tag="ps", name="cp")
                    cp = st["cp"]
                    for nn in range(2):
                        nc.tensor.matmul(
                            cp[0 : D + 1, nn * 512 : (nn + 1) * 512],
                            v_sb[:, j, h * (D + 1) : (h + 1) * (D + 1)],
                            ets[j][:, nn * 512 : (nn + 1) * 512],
                            start=(j == 0),
                            stop=(j == JC - 1),
                        )

                def final_copy():
                    nc.vector.tensor_copy(
                        ctx_sb[0 : D + 1, ihalf * 1024 : (ihalf + 1) * 1024],
                        st["cp"][0 : D + 1, :],
                    )

                for jc in range(JC):
                    sp = psp.tile([128, 1024], f32, tag="ps", name="sp")
                    for nn in range(2):
                        nc.tensor.matmul(
                            sp[:, nn * 512 : (nn + 1) * 512],
                            ka[0:KAUG, jc * 128 : (jc + 1) * 128],
                            qa[0:KAUG, ihalf * 1024 + nn * 512 : ihalf * 1024 + (nn + 1) * 512],
                            start=True,
                            stop=True,
                        )
                    et = expp.tile([128, 1024], f32r, tag="expp")
                    nc.scalar.activation(et[:], sp[:], Exp)
                    ets[jc] = et
                    if jc < len(prev_defer):
                        prev_defer[jc]()
                    # own lagged ctx, one jc behind; the last two are deferred
                    if jc > 0 and jc - 1 <= JC - 3:
                        emit_ctx(jc - 1)
                    if jc >= 2 and fillq:
                        fillq.pop(0)()
                for th in fillq:
                    th()
                return [
                    (lambda: emit_ctx(JC - 2)),
                    (lambda: (emit_ctx(JC - 1), final_copy())),
                ]

            # ---- pair 0 projections: q-half1 deferred into the first section ----
            make_aug(0)
            for th in proj_thunks(0, 0, 0) + proj_thunks(0, 1, 0) + proj_thunks(0, 1, 1):
                th()

            ctx_sbs = {}
            defer = []
            pending_tail = []  # tail thunks of the previous pair's odd head
            carry = proj_thunks(0, 0, 1)  # q half-1: needed only from (h0, ihalf1)
            for hp in range(3):
                if hp + 1 < 3:
                    make_aug(hp + 1)
                    pj = []
                    for t in range(2):
                        for half in range(2):
                            pj += proj_thunks(hp + 1, t, half)
                else:
                    pj = []
                h_e, h_o = 2 * hp, 2 * hp + 1
                ctx_sbs[h_e] = ctxs.tile([D + 1, S], f32, tag="ctx_sb", name="ctx_sb")
                ctx_sbs[h_o] = ctxs.tile([D + 1, S], f32, tag="ctx_sb", name="ctx_sb")

                s0_fill = (carry + pending_tail)[:16]
                rest = (carry + pending_tail)[16:]
                carry = []
                s1_fill = rest + pj[: 16 - len(rest)]
                pj = pj[16 - len(rest) :]

                defer = attn_section(h_e, 0, ctx_sbs[h_e], s0_fill, defer)
                defer = attn_section(h_e, 1, ctx_sbs[h_e], s1_fill, defer)

                te = tail_thunks(h_e, ctx_sbs[h_e])
                s2_fill = te[:16]
                s3_fill = te[16:] + pj
                defer = attn_section(h_o, 0, ctx_sbs[h_o], s2_fill, defer)
                defer = attn_section(h_o, 1, ctx_sbs[h_o], s3_fill, defer)

                pending_tail = tail_thunks(h_o, ctx_sbs[h_o])

            for th in defer:
                th()
            for th in pending_tail:
                th()

    nc.finalize()
    return nc


def _get_nc(use_qk_bias: bool):
    key = ("prog", use_qk_bias)
    if key not in _CACHE:
        _CACHE[key] = _build_program(use_qk_bias)
    return _CACHE[key]


def _get_runner(use_qk_bias: bool):
    """Cached PJRT runner: one jitted executable + resident zero-output
    buffers; per call only the input blob is uploaded."""
    key = ("runner", use_qk_bias)
    if key in _CACHE:
        return _CACHE[key]

    nc = _get_nc(use_qk_bias)

    import jax
    from jax.sharding import Mesh, PartitionSpec
    from jax.experimental.shard_map import shard_map
    import concourse.mybir as mybir
    from concourse.bass2jax import (
        _bass_exec_p,
        install_neuronx_cc_hook,
        partition_id_tensor,
    )

    install_neuronx_cc_hook()
    partition_name = nc.partition_id_tensor.name if nc.partition_id_tensor else None
    in_names, out_names, out_avals, zero_outs = [], [], [], []
    for alloc in nc.m.functions[0].allocations:
        if not isinstance(alloc, mybir.MemoryLocationSet):
            continue
        name = alloc.memorylocations[0].name
        if alloc.kind == "ExternalInput":
            if name != partition_name:
                in_names.append(name)
        elif alloc.kind == "ExternalOutput":
            out_names.append(name)
            shape = tuple(alloc.tensor_shape)
            dtype = mybir.dt.np(alloc.dtype)
            out_avals.append(jax.core.ShapedArray(shape, dtype))
            zero_outs.append(np.zeros(shape, dtype))
    assert in_names == ["blob"] and out_names == ["out"]
    all_in_names = list(in_names) + list(out_names)
    if partition_name is not None:
        all_in_names.append(partition_name)

    def _body(*args):
        operands = list(args)
        if partition_name is not None:
            operands.append(partition_id_tensor())
        return tuple(
            _bass_exec_p.bind(
                *operands,
                out_avals=tuple(out_avals),
                in_names=tuple(all_in_names),
                out_names=tuple(out_names),
                lowering_input_output_aliases=(),
                sim_require_finite=True,
                sim_require_nnan=True,
                nc=nc,
            )
        )

    devices = jax.devices()[:NCORES]
    mesh = Mesh(np.asarray(devices), ("core",))
    sharded = jax.jit(
        shard_map(
            _body,
            mesh=mesh,
            in_specs=(PartitionSpec("core"),) * 2,
            out_specs=(PartitionSpec("core"),),
            check_rep=False,
        ),
        keep_unused=True,
    )
    dev_zeros = jax.device_put(
        np.zeros((NCORES * zero_outs[0].shape[0], *zero_outs[0].shape[1:]), zero_outs[0].dtype)
    )
    out_shape = tuple(out_avals[0].shape)

    def run(blobs: np.ndarray):
        (out,) = sharded(blobs, dev_zeros)
        return np.asarray(out).reshape(NCORES, *out_shape)

    _CACHE[key] = run
    return run


def _make_in_maps(hidden_states, Wq, bq, Wk, bk, Wv, bv, alphas, use_qk_bias):
    """Per-core packed blobs (bf16 1-D), as list of dicts keyed 'blob'."""
    import ml_dtypes

    bf = ml_dtypes.bfloat16
    G, _ = _cheb_factors(alphas)  # [12, 6, S] f32
    scale = np.float32(1.0 / np.sqrt(np.float32(D)))

    hs_b = [
        np.ascontiguousarray(hidden_states[b].T).astype(bf).ravel() for b in range(B)
    ]
    w_hg, g_hg, bqk_hg = [], [], []
    for hg in range(2):
        rows = slice(hg * HGDIM, (hg + 1) * HGDIM)
        w = np.stack(
            [
                np.ascontiguousarray(Wq[rows, :].T) * scale,
                np.ascontiguousarray(Wk[rows, :].T),
                np.ascontiguousarray(Wv[rows, :].T),
            ],
            0,
        ).astype(bf)
        w_hg.append(w.ravel())
        g_hg.append(
            np.ascontiguousarray(G[hg * HG : (hg + 1) * HG], dtype=np.float32)
            .ravel()
            .view(bf)
        )
        bqk_hg.append(
            np.ascontiguousarray(
                np.stack([bq[rows] * scale, bk[rows]], 0), dtype=np.float32
            )
            .ravel()
            .view(bf)
        )

    in_maps = []
    for core in range(NCORES):
        b, hg = divmod(core, 2)
        parts = [hs_b[b], w_hg[hg], g_hg[hg]]
        if use_qk_bias:
            parts.append(bqk_hg[hg])
        in_maps.append({"blob": np.concatenate(parts)})
    return in_maps


def kernel(hidden_states, Wq, bq, Wk, bk, Wv, bv, alphas):
    hidden_states = np.asarray(hidden_states, dtype=np.float32)
    Wq = np.asarray(Wq, dtype=np.float32)
    Wk = np.asarray(Wk, dtype=np.float32)
    Wv = np.asarray(Wv, dtype=np.float32)
    bq = np.asarray(bq, dtype=np.float32)
    bk = np.asarray(bk, dtype=np.float32)
    bv = np.asarray(bv, dtype=np.float32)
    alphas = np.asarray(alphas, dtype=np.float32)

    use_qk_bias = bool(np.any(bq) or np.any(bk))
    run = _get_runner(use_qk_bias)
    in_maps = _make_in_maps(
        hidden_states, Wq, bq, Wk, bk, Wv, bv, alphas, use_qk_bias
    )
    blobs = np.concatenate([m["blob"] for m in in_maps])
    res = run(blobs)  # [8, S, HGDIM] bf16

    out = np.empty((B, S, HIDDEN), dtype=np.float32)
    add_bv = bool(np.any(bv))
    for core in range(NCORES):
        b, hg = divmod(core, 2)
        o = res[core].astype(np.float32)  # [S, 384]
        if add_bv:
            o = o + bv[hg * HGDIM : (hg + 1) * HGDIM][None, :]
        out[b, :, hg * HGDIM : (hg + 1) * HGDIM] = o
    return out
